# revision 1
# baseline (speedup 1.0000x reference)
"""Trainium2 Bass kernel for nn_BaseGR (2-layer hetero-SAGE GNN + predictor).

8-core strategy:
  - Users sharded 12500/core, items sharded 2500/core (padded blocks of
    2560); group rows replicated via partial sums + AllReduce.
  - Each segment-mean: dma_gather of neighbor feature rows (bf16, HBM) ->
    one-hot built on DVE (iota==dst_local)*weight -> TensorE scatter-matmul
    accumulating [H, dst_tile] in PSUM -> W-matmul.
  - User/group tables store BOTH layers' features per 512B row
    ([h0 | h1]), so one gather serves layer 1 and layer 2 (the gather cost
    is per-row latency-bound, so doubling the row size is ~free).
  - oi1 partials are ReduceScattered item-major (rank c receives exactly
    its item shard); og1/og2 partials share one bf16 AllReduce.
  - Final predictor computed transposed ([items, groups]) so pred_b is a
    per-partition bias; host returns a zero-cost .T view.
"""

import sys

sys.path.insert(0, "/opt/trn_rl_repo")

import numpy as np
import ml_dtypes

import concourse.bass as bass
import concourse.bacc as bacc
import concourse.mybir as mybir
import concourse.tile as tile
from concourse.bass_utils import run_bass_kernel_spmd
from concourse.alu_op_type import AluOpType

BF16 = ml_dtypes.bfloat16
F32 = np.float32

NG, NU, NI, H = 5000, 100000, 20000, 128
W = 8
USH = NU // W            # 12500 users per core
USH_P = 12544            # padded (98 tiles)
ISH = NI // W            # 2500 items per core
ISH_P = 2560             # padded (20 tiles)
NI_P = ISH_P * W         # 20480 padded item space
NG_P = 5120              # padded groups (40 tiles)
N_UT = USH_P // 128      # 98 user tiles
N_IT = NI_P // 128       # 160 item tiles (padded space)
N_GT = NG_P // 128       # 40 group tiles
N_IST = ISH_P // 128     # 20 local item tiles
SEG_UG = 16              # chunks per gather call (512B rows)
SEG_SM = 24              # chunks per gather call (256B rows)


def _pad_item(i):
    return (i // ISH) * ISH_P + (i % ISH)


class Dir:
    """One gather/scatter direction. Structure (tiles/segments/chunk counts)
    is shared by all cores; index/weight arrays are per-core."""

    def __init__(self, name, n_dst_tiles, force_all_tiles, seg_chunks):
        self.name = name
        self.n_dst_tiles = n_dst_tiles
        self.force_all_tiles = force_all_tiles
        self.seg_chunks = seg_chunks
        self.tiles = []      # [(tile_id, chunk_ofs, n_chunks)]
        self.segments = []   # [(chunk_start, n_chunks, [tile entries])]
        self.total_chunks = 0
        self.idx = None      # [W, 128, C*8] int16 (16-wrapped, replicated)
        self.dstl = None     # [W, 128, C] bf16
        self.wv = None       # [W, 128, C] bf16

    def build(self, per_core):
        ncore = len(per_core)
        buckets = [[None] * self.n_dst_tiles for _ in range(ncore)]
        for c, (gidx, dst, wgt) in enumerate(per_core):
            t = dst // 128
            order = np.argsort(t, kind="stable")
            t_s = t[order]
            bounds = np.searchsorted(t_s, np.arange(self.n_dst_tiles + 1))
            for ti in range(self.n_dst_tiles):
                sl = order[bounds[ti]:bounds[ti + 1]]
                if len(sl):
                    # ascending gather addresses within the tile: the SDMA
                    # round trips are latency-bound; locality helps row hits
                    buckets[c][ti] = sl[np.argsort(gidx[sl], kind="stable")]
        n_chunks = np.zeros(self.n_dst_tiles, np.int64)
        for ti in range(self.n_dst_tiles):
            mx = max(len(buckets[c][ti]) if buckets[c][ti] is not None else 0
                     for c in range(ncore))
            if mx == 0 and self.force_all_tiles:
                mx = 1
            n_chunks[ti] = (mx + 127) // 128 if mx else 0
        ofs = 0
        seg_start, seg_n, seg_tiles = 0, 0, []
        for ti in range(self.n_dst_tiles):
            nc_t = int(n_chunks[ti])
            if nc_t == 0:
                continue
            if seg_n and seg_n + nc_t > self.seg_chunks:
                self.segments.append((seg_start, seg_n, seg_tiles))
                seg_start, seg_n, seg_tiles = ofs, 0, []
            self.tiles.append((ti, ofs, nc_t))
            seg_tiles.append((ti, ofs, nc_t))
            ofs += nc_t
            seg_n += nc_t
        if seg_n:
            self.segments.append((seg_start, seg_n, seg_tiles))
        self.total_chunks = ofs

        C = self.total_chunks
        self.idx = np.zeros((ncore, 128, C * 8), np.int16)
        self.dstl = np.zeros((ncore, 128, C), F32)
        self.wv = np.zeros((ncore, 128, C), F32)
        for c, (gidx, dst, wgt) in enumerate(per_core):
            i1 = np.zeros(C * 128, np.int16)
            dl = np.zeros(C * 128, F32)
            wv = np.zeros(C * 128, F32)
            for (ti, ofs_t, nct) in self.tiles:
                sl = buckets[c][ti]
                if sl is None:
                    continue
                n = len(sl)
                base = ofs_t * 128
                i1[base:base + n] = gidx[sl]
                dl[base:base + n] = (dst[sl] - ti * 128).astype(F32)
                wv[base:base + n] = wgt[sl]
            for (cs, cn, _st) in self.segments:
                blk = i1[cs * 128:(cs + cn) * 128].reshape(16, cn * 8, order="F")
                self.idx[c][:, cs * 8:(cs + cn) * 8] = np.tile(blk, (8, 1))
            self.dstl[c] = dl.reshape(C, 128).T
            self.wv[c] = wv.reshape(C, 128).T


def _prep(inputs):
    x_user = np.asarray(inputs["x_user"])
    x_item = np.asarray(inputs["x_item"])
    hu0 = np.asarray(inputs["emb_user"], F32)[x_user]
    hi0 = np.asarray(inputs["emb_item"], F32)[x_item]
    W1l = np.asarray(inputs["W1l"], F32)
    W1r = np.asarray(inputs["W1r"], F32)
    b1 = np.asarray(inputs["b1"], F32)
    W2l = np.asarray(inputs["W2l"], F32)
    W2r = np.asarray(inputs["W2r"], F32)
    b2 = np.asarray(inputs["b2"], F32)
    predW = np.asarray(inputs["pred_W"], F32)
    predb = np.asarray(inputs["pred_b"], F32)
    ug_src = np.asarray(inputs["ug_src"], np.int64)
    ug_dst = np.asarray(inputs["ug_dst"], np.int64)
    ui_src = np.asarray(inputs["ui_src"], np.int64)
    ui_dst = np.asarray(inputs["ui_dst"], np.int64)
    gi_src = np.asarray(inputs["gi_src"], np.int64)
    gi_dst = np.asarray(inputs["gi_dst"], np.int64)

    w_ug_g = (1.0 / np.maximum(np.bincount(ug_dst, minlength=NG), 1)).astype(F32)
    w_gi_g = (1.0 / np.maximum(np.bincount(gi_src, minlength=NG), 1)).astype(F32)
    w_ui_i = (1.0 / np.maximum(np.bincount(ui_dst, minlength=NI), 1)).astype(F32)
    w_ui_u = (1.0 / np.maximum(np.bincount(ui_src, minlength=NU), 1)).astype(F32)

    # user table [USH_P, 256]: cols 0:128 = hu0 shard; 128:256 = hu1 (device)
    ugt = np.zeros((W, USH_P, 2 * H), BF16)
    # item shard table [ISH_P, 256]: cols 0:128 = hi0 shard; 128:256 = hi1
    git = np.zeros((W, ISH_P, 2 * H), BF16)
    # full item table (layer1 features only) for i2u gathers
    ite = np.zeros((NI_P, H), BF16)
    for c in range(W):
        ugt[c, :USH, :H] = hu0[c * USH:(c + 1) * USH].astype(BF16)
        git[c, :ISH, :H] = hi0[c * ISH:(c + 1) * ISH].astype(BF16)
        ite[c * ISH_P:c * ISH_P + ISH] = hi0[c * ISH:(c + 1) * ISH].astype(BF16)

    d_ug = Dir("ug", N_GT, False, SEG_UG)
    per = []
    for c in range(W):
        m = (ug_src >= c * USH) & (ug_src < (c + 1) * USH)
        per.append(((ug_src[m] - c * USH).astype(np.int16),
                    ug_dst[m], w_ug_g[ug_dst[m]]))
    d_ug.build(per)

    # gi is dense enough (25K edges onto 2560x5120 per core) that a
    # host-built adjacency block beats per-edge gathers 4x.
    agi = np.zeros((W, ISH_P, NG_P), BF16)
    for c in range(W):
        m = (gi_dst >= c * ISH) & (gi_dst < (c + 1) * ISH)
        il = (gi_dst[m] - c * ISH).astype(np.int64)
        g = gi_src[m]
        acc = np.zeros((ISH_P, NG_P), F32)
        np.add.at(acc, (il, g), w_gi_g[g])
        agi[c] = acc.astype(BF16)

    d_uii = Dir("uii", N_IT, True, SEG_SM)   # u2i: dst = items (padded)
    d_iu = Dir("iu", N_UT, True, SEG_SM)     # i2u: dst = local users
    per_uii, per_iu = [], []
    for c in range(W):
        m = (ui_src >= c * USH) & (ui_src < (c + 1) * USH)
        us, ud = ui_src[m], ui_dst[m]
        per_uii.append(((us - c * USH).astype(np.int16),
                        _pad_item(ud), w_ui_i[ud]))
        per_iu.append((_pad_item(ud).astype(np.int16),
                       (us - c * USH), w_ui_u[us]))
    d_uii.build(per_uii)
    d_iu.build(per_iu)

    wts = np.stack([
        W1l[0], W1l[5],                 # og1: u2g, i2g
        W1l[2], W1r[2] + W1r[4],        # oi1: u2i agg, dense
        W1l[3], W1r[1] + W1r[3],        # ou1: i2u agg, dense
        W2l[0], W2l[5], W2r[0] + W2r[5]  # og2
    ]).astype(BF16)
    biases = np.stack([b1[0] + b1[5], b1[1] + b1[3],
                       b2[0] + b2[5], np.zeros(H, F32)], axis=1).astype(F32)
    btile_i1 = np.broadcast_to((b1[2] + b1[4]).astype(BF16), (128, H)).copy()
    ident = np.eye(128, dtype=BF16)
    iota = np.broadcast_to(np.arange(128, dtype=F32), (128, 128)).copy()

    predW_sh = np.zeros((W, H, ISH_P), BF16)
    predb_sh = np.zeros((W, N_IST, 128), F32)
    for c in range(W):
        predW_sh[c][:, :ISH] = predW[:, c * ISH:(c + 1) * ISH].astype(BF16)
        pb = np.zeros(ISH_P, F32)
        pb[:ISH] = predb[c * ISH:(c + 1) * ISH]
        predb_sh[c] = pb.reshape(N_IST, 128)

    in_maps = []
    for c in range(W):
        in_maps.append({
            "ugt": ugt[c], "git": git[c], "ite": ite,
            "wts": wts, "biases": biases, "btile_i1": btile_i1,
            "ident": ident, "iota": iota,
            "predw": predW_sh[c], "predb": predb_sh[c],
            "ug_idx": d_ug.idx[c], "ug_dstl": d_ug.dstl[c], "ug_wv": d_ug.wv[c],
            "agi": agi[c],
            "uii_idx": d_uii.idx[c], "uii_dstl": d_uii.dstl[c],
            "uii_wv": d_uii.wv[c],
            "iu_idx": d_iu.idx[c], "iu_dstl": d_iu.dstl[c], "iu_wv": d_iu.wv[c],
        })
    return in_maps, {"ug": d_ug, "uii": d_uii, "iu": d_iu}


def _build(struct):
    d_ug, d_uii, d_iu = struct["ug"], struct["uii"], struct["iu"]
    nc = bacc.Bacc("TRN2", target_bir_lowering=False)
    bf = mybir.dt.bfloat16
    f32 = mybir.dt.float32
    i16 = mybir.dt.int16

    P = {}

    def param(name, shape, dt):
        P[name] = nc.declare_dram_parameter(name, list(shape), dt,
                                            isOutput=False)
        return P[name]

    ugt = param("ugt", [USH_P, 2 * H], bf)
    git = param("git", [ISH_P, 2 * H], bf)
    ite = param("ite", [NI_P, H], bf)
    wts = param("wts", [9, 128, 128], bf)
    biases = param("biases", [128, 4], f32)
    btile_i1 = param("btile_i1", [128, H], bf)
    ident_d = param("ident", [128, 128], bf)
    iota_d = param("iota", [128, 128], f32)
    predw = param("predw", [H, ISH_P], bf)
    predb = param("predb", [N_IST, 128], f32)
    agi_d = param("agi", [ISH_P, NG_P], bf)
    for nm, d in (("ug", d_ug), ("uii", d_uii), ("iu", d_iu)):
        C = d.total_chunks
        param(f"{nm}_idx", [128, C * 8], i16)
        param(f"{nm}_dstl", [128, C], f32)
        param(f"{nm}_wv", [128, C], f32)
    outp = nc.declare_dram_parameter("out", [ISH_P, NG], bf, isOutput=True)

    with tile.TileContext(nc) as tc:
        with (
            tc.tile_pool(name="cst", bufs=1) as cst,
            tc.tile_pool(name="gp", bufs=2) as gp,
            tc.tile_pool(name="sp", bufs=3) as sp,
            tc.tile_pool(name="st", bufs=2) as stp,
            tc.tile_pool(name="big", bufs=2) as bigp,
            tc.tile_pool(name="psum", bufs=1, space="PSUM") as psum,
            tc.tile_pool(name="dram", bufs=1, space="DRAM") as dram,
        ):
            wt_sb = []
            for k in range(9):
                t = cst.tile([128, 128], bf, tag=f"w{k}")
                nc.sync.dma_start(t[:], wts[k])
                wt_sb.append(t)
            (W_og_u, W_og_i, W_oi_a, W_oi_d, W_ou_a, W_ou_d,
             W_og2_u, W_og2_i, W_og2_d) = wt_sb
            bias_sb = cst.tile([128, 4], f32, tag="bias")
            nc.sync.dma_start(bias_sb[:], biases[:])
            bti_sb = cst.tile([128, H], bf, tag="bti")
            nc.sync.dma_start(bti_sb[:], btile_i1[:])
            ident_sb = cst.tile([128, 128], bf, tag="ident")
            nc.sync.dma_start(ident_sb[:], ident_d[:])
            predb_sb = cst.tile([128, N_IST], f32, tag="predb")
            nc.sync.dma_start(predb_sb[:], predb[:].rearrange("a b -> b a"))
            iota_sb = cst.tile([128, 128], f32, tag="iota")
            nc.sync.dma_start(iota_sb[:], iota_d[:])

            darr = {}
            for nm, d in (("ug", d_ug), ("uii", d_uii), ("iu", d_iu)):
                C = d.total_chunks
                ti_ = cst.tile([128, C * 8], i16, tag=f"{nm}_idx")
                nc.sync.dma_start(ti_[:], P[f"{nm}_idx"][:])
                td = cst.tile([128, C], f32, tag=f"{nm}_dstl")
                nc.sync.dma_start(td[:], P[f"{nm}_dstl"][:])
                tw = cst.tile([128, C], f32, tag=f"{nm}_wv")
                nc.sync.dma_start(tw[:], P[f"{nm}_wv"][:])
                darr[nm] = (ti_, td, tw)

            ogT = bigp.tile([128, 2 * NG_P], bf, tag="big", name="ogT")
            nc.vector.memset(ogT[:], 0.0)
            og1T = ogT[:, 0:NG_P]
            og2T = ogT[:, NG_P:2 * NG_P]
            hg1T = cst.tile([128, NG_P], bf, tag="hg1T")
            repT = cst.tile([128, NG_P], bf, tag="repT")

            hiT_full = cst.tile([128, ISH_P], bf, tag="hiTf")
            nc.sync.dma_start(hiT_full[:], git[:, 0:H], transpose=True)

            aroi_in = dram.tile([NI_P, H], bf)
            rs_oi = dram.tile([ISH_P, H], bf)
            ar_in = dram.tile([128, 2 * NG_P], bf)
            ar_out = dram.tile([128, 2 * NG_P], bf)

            def segsum(d, table_ap, elem_size, elem_step, width, out_cb):
                idx_sb, dstl_sb, wv_sb = darr[d.name]
                for (cs, cn, seg_tiles) in d.segments:
                    gt = gp.tile([128, d.seg_chunks, width], bf,
                                 tag=f"gath{width}",
                                 bufs=(3 if width == H else 2))
                    n_idx = cn * 128
                    nc.gpsimd.dma_gather(
                        gt[:, :cn, :], table_ap,
                        idx_sb[:, cs * 8:(cs + cn) * 8],
                        n_idx, n_idx, elem_size, elem_step=elem_step,
                        single_packet=False)
                    oh = gp.tile([128, d.seg_chunks, 128], bf, tag="oh")
                    iota_b = (iota_sb[:].rearrange("p (o k) -> p o k", o=1)
                              .to_broadcast((128, cn, 128)))
                    dstl_b = (dstl_sb[:, cs:cs + cn]
                              .rearrange("p (c o) -> p c o", o=1)
                              .to_broadcast((128, cn, 128)))
                    wv_b = (wv_sb[:, cs:cs + cn]
                            .rearrange("p (c o) -> p c o", o=1)
                            .to_broadcast((128, cn, 128)))
                    ohq = gp.tile([128, d.seg_chunks, 128], bf, tag="ohq")
                    nc.vector.tensor_tensor(ohq[:, :cn, :], iota_b, dstl_b,
                                            AluOpType.is_equal)
                    nc.vector.tensor_tensor(oh[:, :cn, :], ohq[:, :cn, :],
                                            wv_b, AluOpType.mult)
                    for (ti, ofs_t, nct) in seg_tiles:
                        out_cb(ti, gt, oh, ofs_t - cs, nct)

            # ---------- Phase 1: i2u -> hu1 (local users) ----------
            hu_stage = [None]

            huTg_cache = [None]

            def get_huT(ti):
                g8 = ti // 8
                if huTg_cache[0] is None or huTg_cache[0][0] != g8:
                    n_t = min(8, N_UT - g8 * 8)
                    tl = sp.tile([128, 1024], bf, tag="huTg", name="huTg", bufs=2)
                    nc.sync.dma_start(
                        tl[:, :n_t * 128],
                        ugt[g8 * 1024:g8 * 1024 + n_t * 128, 0:H],
                        transpose=True)
                    huTg_cache[0] = (g8, tl)
                return huTg_cache[0][1][:, (ti % 8) * 128:(ti % 8 + 1) * 128]

            def cb_ou(ti, gt, oh, lc0, nct):
                ps = psum.tile([128, 128], f32, tag="agg", bufs=2)
                for j in range(nct):
                    nc.tensor.matmul(ps[:], gt[:, lc0 + j, :], oh[:, lc0 + j, :],
                                     start=(j == 0), stop=(j == nct - 1))
                aggT = sp.tile([128, 128], bf, tag="aggT", bufs=4)
                nc.scalar.activation(aggT[:], ps[:],
                                     mybir.ActivationFunctionType.Copy)
                pw = psum.tile([128, 128], f32, tag="w", bufs=2)
                nc.tensor.matmul(pw[:], W_ou_a[:], aggT[:], start=True,
                                 stop=False)
                nc.tensor.matmul(pw[:], W_ou_d[:], get_huT(ti), start=False,
                                 stop=True)
                ouT = sp.tile([128, 128], bf, tag="ouT", bufs=4)
                nc.scalar.activation(ouT[:], pw[:],
                                     mybir.ActivationFunctionType.Relu,
                                     bias=bias_sb[:, 1:2])
                ptr = psum.tile([128, 128], bf, tag="w", bufs=2)
                nc.tensor.transpose(ptr[:], ouT[:], ident_sb[:])
                g, s = ti // 16, ti % 16
                if hu_stage[0] is None:
                    hu_stage[0] = stp.tile([128, 16, 128], bf, tag="hust", name="hust")
                nc.vector.tensor_copy(hu_stage[0][:, s, :], ptr[:])
                if s == 15 or ti == N_UT - 1:
                    n_g = s + 1
                    nc.sync.dma_start(
                        ugt[g * 2048:g * 2048 + n_g * 128, H:2 * H]
                        .rearrange("(k p) h -> p k h", p=128),
                        hu_stage[0][:, :n_g, :])
                    hu_stage[0] = None

            segsum(d_iu, ite[:], H, H, H, cb_ou)

            # ---------- Phase 2: u2i -> oi1 partial (item-major) ----------
            oi_stage = [None]

            def cb_oi(ti, gt, oh, lc0, nct):
                ps = psum.tile([128, 128], f32, tag="agg", bufs=2)
                for j in range(nct):
                    nc.tensor.matmul(ps[:], gt[:, lc0 + j, :], oh[:, lc0 + j, :],
                                     start=(j == 0), stop=(j == nct - 1))
                aggT = sp.tile([128, 128], bf, tag="aggT", bufs=4)
                nc.scalar.activation(aggT[:], ps[:],
                                     mybir.ActivationFunctionType.Copy)
                pw = psum.tile([128, 128], f32, tag="w", bufs=2)
                nc.tensor.matmul(pw[:], aggT[:], W_oi_a[:], start=True,
                                 stop=True)
                g, s = ti // 16, ti % 16
                if oi_stage[0] is None:
                    oi_stage[0] = stp.tile([128, 16, 128], bf, tag="oist", name="oist")
                nc.vector.tensor_copy(oi_stage[0][:, s, :], pw[:])
                if s == 15 or ti == N_IT - 1:
                    n_g = s + 1
                    nc.sync.dma_start(
                        aroi_in[g * 2048:g * 2048 + n_g * 128, :]
                        .rearrange("(k p) h -> p k h", p=128),
                        oi_stage[0][:, :n_g, :])
                    oi_stage[0] = None

            segsum(d_uii, ugt[:, 0:H], H, 2 * H, H, cb_oi)

            nc.gpsimd.collective_compute(
                "ReduceScatter", AluOpType.add,
                replica_groups=[list(range(W))],
                ins=[aroi_in.opt()], outs=[rs_oi.opt()])

            # ---------- Phases 4a: u2g both layers (needs hu1 only) ------
            def make_cb_g(W_l1, W_l2):
                def cb(ti, gt, oh, lc0, nct):
                    ps0 = psum.tile([128, 128], f32, tag="agg", bufs=2)
                    ps1 = psum.tile([128, 128], f32, tag="agg1", bufs=2)
                    for j in range(nct):
                        nc.tensor.matmul(ps0[:], gt[:, lc0 + j, 0:H],
                                         oh[:, lc0 + j, :],
                                         start=(j == 0), stop=(j == nct - 1))
                        nc.tensor.matmul(ps1[:], gt[:, lc0 + j, H:2 * H],
                                         oh[:, lc0 + j, :],
                                         start=(j == 0), stop=(j == nct - 1))
                    a0 = sp.tile([128, 128], bf, tag="aggT", bufs=4)
                    nc.scalar.activation(a0[:], ps0[:],
                                         mybir.ActivationFunctionType.Copy)
                    a1 = sp.tile([128, 128], bf, tag="aggT2", bufs=4)
                    nc.scalar.activation(a1[:], ps1[:],
                                         mybir.ActivationFunctionType.Copy)
                    pw = psum.tile([128, 128], f32, tag="w", bufs=2)
                    nc.tensor.matmul(pw[:], W_l1[:], a0[:], start=True,
                                     stop=True)
                    sl = slice(ti * 128, (ti + 1) * 128)
                    nc.vector.tensor_tensor(og1T[:, sl], og1T[:, sl], pw[:],
                                            AluOpType.add)
                    pw2 = psum.tile([128, 128], f32, tag="w", bufs=2)
                    nc.tensor.matmul(pw2[:], W_l2[:], a1[:], start=True,
                                     stop=True)
                    nc.vector.tensor_tensor(og2T[:, sl], og2T[:, sl], pw2[:],
                                            AluOpType.add)
                return cb

            segsum(d_ug, ugt[:], 2 * H, 2 * H, 2 * H,
                   make_cb_g(W_og_u, W_og2_u))

            # ---------- Phase 3: hi1 = relu(rs + dense + b) ----------
            for t in range(N_IST):
                rs_sb = sp.tile([128, 128], bf, tag="rs")
                nc.sync.dma_start(rs_sb[:], rs_oi[t * 128:(t + 1) * 128, :])
                pd = psum.tile([128, 128], f32, tag="w", bufs=2)
                nc.tensor.matmul(pd[:], hiT_full[:, t * 128:(t + 1) * 128],
                                 W_oi_d[:], start=True, stop=True)
                t1 = sp.tile([128, 128], bf, tag="t1")
                nc.vector.tensor_tensor(t1[:], rs_sb[:], pd[:], AluOpType.add)
                t2 = sp.tile([128, 128], bf, tag="t2")
                nc.vector.tensor_tensor(t2[:], t1[:], bti_sb[:], AluOpType.add)
                hi1_t = sp.tile([128, 128], bf, tag="hi1")
                nc.scalar.activation(hi1_t[:], t2[:],
                                     mybir.ActivationFunctionType.Relu)
                nc.sync.dma_start(git[t * 128:(t + 1) * 128, H:2 * H], hi1_t[:])

            # ---------- Phase 5: i2g both layers via dense adjacency ----
            # mean_T[H, groups] = sum_t git_tile[K=item,H].T @ A[item, groups];
            # then (mean @ W) folds in afterwards per 1024-col block.
            for jg in range(NG_P // 1024):
                p0 = psum.tile([128, 1024], f32, tag="agg", bufs=2)
                p1 = psum.tile([128, 1024], f32, tag="agg", bufs=2)
                for t in range(N_IST):
                    gsb = sp.tile([128, 2 * H], bf, tag="gisb", bufs=2)
                    nc.sync.dma_start(gsb[:], git[t * 128:(t + 1) * 128, :])
                    asb = sp.tile([128, 1024], bf, tag="agisb", bufs=2)
                    nc.sync.dma_start(
                        asb[:],
                        agi_d[t * 128:(t + 1) * 128,
                              jg * 1024:(jg + 1) * 1024])
                    for q in range(2):
                        nc.tensor.matmul(
                            p0[:, q * 512:(q + 1) * 512], gsb[:, 0:H],
                            asb[:, q * 512:(q + 1) * 512],
                            start=(t == 0), stop=(t == N_IST - 1))
                        nc.tensor.matmul(
                            p1[:, q * 512:(q + 1) * 512], gsb[:, H:2 * H],
                            asb[:, q * 512:(q + 1) * 512],
                            start=(t == 0), stop=(t == N_IST - 1))
                # aggregate-T is now in psum; fold W via aggT copy + W-MM
                for k in range(8):
                    sl = slice(jg * 1024 + k * 128, jg * 1024 + (k + 1) * 128)
                    a0 = sp.tile([128, 128], bf, tag="aggT", bufs=4)
                    nc.scalar.activation(a0[:], p0[:, k * 128:(k + 1) * 128],
                                         mybir.ActivationFunctionType.Copy)
                    a1 = sp.tile([128, 128], bf, tag="aggT2", bufs=4)
                    nc.scalar.activation(a1[:], p1[:, k * 128:(k + 1) * 128],
                                         mybir.ActivationFunctionType.Copy)
                    pw = psum.tile([128, 128], f32, tag="w", bufs=2)
                    nc.tensor.matmul(pw[:], W_og_i[:], a0[:], start=True,
                                     stop=True)
                    nc.vector.tensor_tensor(og1T[:, sl], og1T[:, sl], pw[:],
                                            AluOpType.add)
                    pw2 = psum.tile([128, 128], f32, tag="w", bufs=2)
                    nc.tensor.matmul(pw2[:], W_og2_i[:], a1[:], start=True,
                                     stop=True)
                    nc.vector.tensor_tensor(og2T[:, sl], og2T[:, sl], pw2[:],
                                            AluOpType.add)

            # ---------- Phase 6: AllReduce og1|og2, activations ----------
            nc.sync.dma_start(ar_in[:], ogT[:])
            nc.gpsimd.collective_compute(
                "AllReduce", AluOpType.add,
                replica_groups=[list(range(W))],
                ins=[ar_in.opt()], outs=[ar_out.opt()])
            ar_sb = bigp.tile([128, 2 * NG_P], bf, tag="big", name="ar_sb")
            nc.sync.dma_start(ar_sb[:], ar_out[:])
            nc.scalar.activation(hg1T[:], ar_sb[:, 0:NG_P],
                                 mybir.ActivationFunctionType.Relu,
                                 bias=bias_sb[:, 0:1])
            for j in range(NG_P // 512):
                pf = psum.tile([128, 512], f32, tag="agg", bufs=2)
                nc.tensor.matmul(pf[:], W_og2_d[:],
                                 hg1T[:, j * 512:(j + 1) * 512],
                                 start=True, stop=True)
                tt = sp.tile([128, 512], bf, tag="o2t")
                nc.vector.tensor_tensor(
                    tt[:], ar_sb[:, NG_P + j * 512:NG_P + (j + 1) * 512],
                    pf[:], AluOpType.add)
                nc.scalar.activation(repT[:, j * 512:(j + 1) * 512], tt[:],
                                     mybir.ActivationFunctionType.Relu,
                                     bias=bias_sb[:, 2:3])

            # ---------- Phase 7: out[item, group] = predW.T @ repT + b ----
            for t in range(N_IST):
                pw_t = sp.tile([H, 128], bf, tag="pwt")
                nc.sync.dma_start(pw_t[:], predw[:, t * 128:(t + 1) * 128])
                for j in range((NG + 1023) // 1024):
                    wj = min(1024, NG - j * 1024)
                    pf = psum.tile([128, 1024], f32, tag="agg", bufs=2)
                    for q in range((wj + 511) // 512):
                        wq = min(512, wj - q * 512)
                        col = j * 1024 + q * 512
                        nc.tensor.matmul(
                            pf[:, q * 512:q * 512 + wq],
                            pw_t[:],
                            repT[:, col:col + wq], start=True, stop=True)
                    stg = stp.tile([128, 1024], bf, tag="fstage", bufs=3)
                    nc.vector.tensor_scalar(
                        stg[:, :wj], pf[:, :wj],
                        predb_sb[:, t:t + 1], None, AluOpType.add)
                    nc.sync.dma_start(
                        outp[t * 128:(t + 1) * 128, j * 1024:j * 1024 + wj],
                        stg[:, :wj])
    nc.compile()
    return nc


def kernel(**inputs):
    in_maps, struct = _prep(inputs)
    nc = _build(struct)
    res = run_bass_kernel_spmd(nc, in_maps, list(range(W)))
    parts = [res.results[c]["out"][:ISH] for c in range(W)]
    full = np.concatenate(parts, axis=0).astype(np.float32)  # [NI, NG]
    return full.T  # [NG, NI] zero-copy view



# revision 7
# speedup vs baseline: 1.9295x; 1.9295x over previous
"""Trainium2 Bass kernel for nn_BaseGR (2-layer hetero-SAGE GNN + predictor).

8-core strategy (v2 — host-pregathered edge tables, single device gather):
  - Users sharded 12500/core, items sharded 2500/core, groups replicated
    via partial og sums + one AllReduce of [og1 | og2].
  - All LAYER-1 aggregations operate on layer-0 features (hu0/hi0), which
    are host-known: the host pre-gathers them into contiguous,
    dst-tile-sorted edge-feature tables ([128, C, H] partition-major).
    The device streams these at HWDGE line rate and scatter-sums via
    one-hot matmuls -- zero Q7/SWDGE descriptor generation.
  - u2i is dst(item)-sharded, so oi1/hi1 are core-local: no ReduceScatter.
  - The ONLY device gather is layer-2's u2g (hu1 rows, device-computed),
    sorted by group tile; i2g layer-2 uses a dense [items_local, NG_P]
    adjacency matmul with hi1 kept in SBUF.
  - Final predictor computed transposed ([items, groups]); host returns a
    zero-cost .T view.
"""

import sys

sys.path.insert(0, "/opt/trn_rl_repo")

import numpy as np
import ml_dtypes

import concourse.bass as bass
import concourse.bacc as bacc
import concourse.mybir as mybir
import concourse.tile as tile
from concourse.bass_utils import run_bass_kernel_spmd
from concourse.alu_op_type import AluOpType

BF16 = ml_dtypes.bfloat16
F32 = np.float32

NG, NU, NI, H = 5000, 100000, 20000, 128
W = 8
USH = NU // W            # 12500 users per core
USH_P = 12544            # padded (98 tiles)
ISH = NI // W            # 2500 items per core
ISH_P = 2560             # padded (20 tiles)
NG_P = 5120              # padded groups (40 tiles)
N_UT = USH_P // 128      # 98 user tiles
N_IST = ISH_P // 128     # 20 local item tiles
N_GT = NG_P // 128       # 40 group tiles
SEG = 32                 # stream segment size (chunks of 128 rows)
SEG_G = 24               # gather segment size


class SDir:
    """A streamed (host-pregathered) scatter direction. Chunk structure is
    shared across cores; tables are per-core."""

    def __init__(self, name, n_dst_tiles):
        self.name = name
        self.n_dst_tiles = n_dst_tiles
        self.tiles = []        # [(ti, chunk_ofs, n_chunks)]
        self.segments = []     # [(cs, cn, [(ti, lc0, nct, done_before, total)])]
        self.total_chunks = 0
        self.tb = None         # [W, 128, C, H] bf16
        self.dstl = None       # [W, 128, C] f32
        self.wv = None         # [W, 128, C] f32

    def build(self, per_core, feat_per_core):
        ncore = len(per_core)
        buckets = [[None] * self.n_dst_tiles for _ in range(ncore)]
        for c, (gidx, dst, wgt) in enumerate(per_core):
            t = dst // 128
            order = np.argsort(t, kind="stable")
            t_s = t[order]
            bounds = np.searchsorted(t_s, np.arange(self.n_dst_tiles + 1))
            for ti in range(self.n_dst_tiles):
                sl = order[bounds[ti]:bounds[ti + 1]]
                buckets[c][ti] = sl
        n_chunks = np.zeros(self.n_dst_tiles, np.int64)
        for ti in range(self.n_dst_tiles):
            mx = max(len(buckets[c][ti]) for c in range(ncore))
            n_chunks[ti] = max((mx + 127) // 128, 1)
        ofs = 0
        for ti in range(self.n_dst_tiles):
            nct = int(n_chunks[ti])
            self.tiles.append((ti, ofs, nct))
            ofs += nct
        self.total_chunks = C = ofs

        # segments: fixed SEG-chunk windows; tiles may straddle windows.
        for cs in range(0, C, SEG):
            cn = min(SEG, C - cs)
            pieces = []
            for (ti, ofs_t, nct) in self.tiles:
                lo = max(ofs_t, cs)
                hi = min(ofs_t + nct, cs + cn)
                if lo < hi:
                    pieces.append((ti, lo - cs, hi - lo, lo - ofs_t, nct))
            self.segments.append((cs, cn, pieces))

        self.tb = np.zeros((ncore, 128, C, H), BF16)
        self.dstl = np.zeros((ncore, 128, C), BF16)
        self.wv = np.zeros((ncore, 128, C), BF16)
        for c, (gidx, dst, wgt) in enumerate(per_core):
            rows = np.zeros((C * 128, H), BF16)
            dl = np.zeros(C * 128, F32)
            wvf = np.zeros(C * 128, F32)
            for (ti, ofs_t, nct) in self.tiles:
                sl = buckets[c][ti]
                n = len(sl)
                if n == 0:
                    continue
                base = ofs_t * 128
                rows[base:base + n] = feat_per_core[c][gidx[sl]]
                dl[base:base + n] = (dst[sl] - ti * 128).astype(F32)
                wvf[base:base + n] = wgt[sl]
            self.tb[c] = rows.reshape(C, 128, H).transpose(1, 0, 2)
            self.dstl[c] = dl.reshape(C, 128).T
            self.wv[c] = wvf.reshape(C, 128).T


class GDir:
    """Device-gather direction (layer-2 u2g over hu1). Same as v1 Dir."""

    def __init__(self, name, n_dst_tiles, seg_chunks):
        self.name = name
        self.n_dst_tiles = n_dst_tiles
        self.seg_chunks = seg_chunks
        self.tiles = []
        self.segments = []   # [(cs, cn, [(ti, ofs_t, nct)])]
        self.total_chunks = 0
        self.idx = None      # [W, 128, C*8] int16
        self.dstl = None
        self.wv = None

    def build(self, per_core):
        ncore = len(per_core)
        buckets = [[None] * self.n_dst_tiles for _ in range(ncore)]
        for c, (gidx, dst, wgt) in enumerate(per_core):
            t = dst // 128
            order = np.argsort(t, kind="stable")
            t_s = t[order]
            bounds = np.searchsorted(t_s, np.arange(self.n_dst_tiles + 1))
            for ti in range(self.n_dst_tiles):
                sl = order[bounds[ti]:bounds[ti + 1]]
                if len(sl):
                    buckets[c][ti] = sl[np.argsort(gidx[sl], kind="stable")]
        n_chunks = np.zeros(self.n_dst_tiles, np.int64)
        for ti in range(self.n_dst_tiles):
            mx = max(len(buckets[c][ti]) if buckets[c][ti] is not None else 0
                     for c in range(ncore))
            n_chunks[ti] = max((mx + 127) // 128, 1)
        ofs = 0
        seg_start, seg_n, seg_tiles = 0, 0, []
        for ti in range(self.n_dst_tiles):
            nct = int(n_chunks[ti])
            if seg_n and seg_n + nct > self.seg_chunks:
                self.segments.append((seg_start, seg_n, seg_tiles))
                seg_start, seg_n, seg_tiles = ofs, 0, []
            self.tiles.append((ti, ofs, nct))
            seg_tiles.append((ti, ofs, nct))
            ofs += nct
            seg_n += nct
        if seg_n:
            self.segments.append((seg_start, seg_n, seg_tiles))
        self.total_chunks = C = ofs

        self.idx = np.zeros((ncore, 128, C * 8), np.int16)
        self.dstl = np.zeros((ncore, 128, C), BF16)
        self.wv = np.zeros((ncore, 128, C), BF16)
        for c, (gidx, dst, wgt) in enumerate(per_core):
            i1 = np.zeros(C * 128, np.int16)
            dl = np.zeros(C * 128, F32)
            wvf = np.zeros(C * 128, F32)
            for (ti, ofs_t, nct) in self.tiles:
                sl = buckets[c][ti]
                if sl is None:
                    continue
                n = len(sl)
                base = ofs_t * 128
                i1[base:base + n] = gidx[sl]
                dl[base:base + n] = (dst[sl] - ti * 128).astype(F32)
                wvf[base:base + n] = wgt[sl]
            for (cs, cn, _st) in self.segments:
                blk = i1[cs * 128:(cs + cn) * 128].reshape(16, cn * 8,
                                                           order="F")
                self.idx[c][:, cs * 8:(cs + cn) * 8] = np.tile(blk, (8, 1))
            self.dstl[c] = dl.reshape(C, 128).T
            self.wv[c] = wvf.reshape(C, 128).T


def _prep(inputs):
    x_user = np.asarray(inputs["x_user"])
    x_item = np.asarray(inputs["x_item"])
    hu0 = np.asarray(inputs["emb_user"], F32)[x_user]
    hi0 = np.asarray(inputs["emb_item"], F32)[x_item]
    W1l = np.asarray(inputs["W1l"], F32)
    W1r = np.asarray(inputs["W1r"], F32)
    b1 = np.asarray(inputs["b1"], F32)
    W2l = np.asarray(inputs["W2l"], F32)
    W2r = np.asarray(inputs["W2r"], F32)
    b2 = np.asarray(inputs["b2"], F32)
    predW = np.asarray(inputs["pred_W"], F32)
    predb = np.asarray(inputs["pred_b"], F32)
    ug_src = np.asarray(inputs["ug_src"], np.int64)
    ug_dst = np.asarray(inputs["ug_dst"], np.int64)
    ui_src = np.asarray(inputs["ui_src"], np.int64)
    ui_dst = np.asarray(inputs["ui_dst"], np.int64)
    gi_src = np.asarray(inputs["gi_src"], np.int64)
    gi_dst = np.asarray(inputs["gi_dst"], np.int64)

    w_ug_g = (1.0 / np.maximum(np.bincount(ug_dst, minlength=NG), 1)).astype(F32)
    w_gi_g = (1.0 / np.maximum(np.bincount(gi_src, minlength=NG), 1)).astype(F32)
    w_ui_i = (1.0 / np.maximum(np.bincount(ui_dst, minlength=NI), 1)).astype(F32)
    w_ui_u = (1.0 / np.maximum(np.bincount(ui_src, minlength=NU), 1)).astype(F32)

    hu0b = hu0.astype(BF16)
    hi0b = hi0.astype(BF16)

    # i2u: dst = users (local), rows = hi0[item]
    d_iu = SDir("iu", N_UT)
    per = []
    for c in range(W):
        m = (ui_src >= c * USH) & (ui_src < (c + 1) * USH)
        per.append((ui_dst[m], ui_src[m] - c * USH, w_ui_u[ui_src[m]]))
    d_iu.build(per, [hi0b] * W)

    # u2i: dst = items (local), rows = hu0[user]
    d_ui = SDir("ui", N_IST)
    per = []
    for c in range(W):
        m = (ui_dst >= c * ISH) & (ui_dst < (c + 1) * ISH)
        per.append((ui_src[m], ui_dst[m] - c * ISH, w_ui_i[ui_dst[m]]))
    d_ui.build(per, [hu0b] * W)

    # u2g layer1: src-user sharded, dst = groups (full range), rows = hu0
    d_ug1 = SDir("ug1", N_GT)
    per = []
    for c in range(W):
        m = (ug_src >= c * USH) & (ug_src < (c + 1) * USH)
        per.append((ug_src[m], ug_dst[m], w_ug_g[ug_dst[m]]))
    d_ug1.build(per, [hu0b] * W)

    # i2g layer1: src-item sharded, dst = groups, rows = hi0
    d_gi1 = SDir("gi1", N_GT)
    per = []
    for c in range(W):
        m = (gi_dst >= c * ISH) & (gi_dst < (c + 1) * ISH)
        per.append((gi_dst[m], gi_src[m], w_gi_g[gi_src[m]]))
    d_gi1.build(per, [hi0b] * W)

    # u2g layer2: gather hu1 rows (local user idx), dst = groups
    d_ug2 = GDir("ug2", N_GT, SEG_G)
    per = []
    for c in range(W):
        m = (ug_src >= c * USH) & (ug_src < (c + 1) * USH)
        per.append(((ug_src[m] - c * USH).astype(np.int16),
                    ug_dst[m], w_ug_g[ug_dst[m]]))
    d_ug2.build(per)

    # i2g layer2 dense adjacency: [ISH_P, NG_P], values w_gi_g
    agi = np.zeros((W, ISH_P, NG_P), BF16)
    for c in range(W):
        m = (gi_dst >= c * ISH) & (gi_dst < (c + 1) * ISH)
        il = (gi_dst[m] - c * ISH).astype(np.int64)
        g = gi_src[m]
        acc = np.zeros((ISH_P, NG_P), F32)
        np.add.at(acc, (il, g), w_gi_g[g])
        agi[c] = acc.astype(BF16)

    # weights: order [W_ou_a, W_ou_d, W_oi_a, W_oi_d, W_og1_u, W_og1_i,
    #                 W_og2_u, W_og2_i, W_og2_d]
    wts = np.stack([
        W1l[3], W1r[1] + W1r[3],
        W1l[2], W1r[2] + W1r[4],
        W1l[0], W1l[5],
        W2l[0], W2l[5], W2r[0] + W2r[5],
    ]).astype(BF16)
    # biases cols: [b_og1, b_ou, b_og2, b_oi]
    biases = np.stack([b1[0] + b1[5], b1[1] + b1[3],
                       b2[0] + b2[5], b1[2] + b1[4]], axis=1).astype(F32)
    ident = np.eye(128, dtype=BF16)
    iota = np.broadcast_to(np.arange(128, dtype=BF16), (128, 128)).copy()

    hu0T = np.zeros((W, 128, USH_P), BF16)
    hi0T = np.zeros((W, 128, ISH_P), BF16)
    for c in range(W):
        hu0T[c][:, :USH] = hu0b[c * USH:(c + 1) * USH].T
        hi0T[c][:, :ISH] = hi0b[c * ISH:(c + 1) * ISH].T

    predW_sh = np.zeros((W, H, ISH_P), BF16)
    predb_sh = np.zeros((W, N_IST, 128), F32)
    for c in range(W):
        predW_sh[c][:, :ISH] = predW[:, c * ISH:(c + 1) * ISH].astype(BF16)
        pb = np.zeros(ISH_P, F32)
        pb[:ISH] = predb[c * ISH:(c + 1) * ISH]
        predb_sh[c] = pb.reshape(N_IST, 128)

    in_maps = []
    for c in range(W):
        mp = {
            "wts": wts, "biases": biases, "ident": ident, "iota": iota,
            "hu0T": hu0T[c], "hi0T": hi0T[c], "agi": agi[c],
            "predw": predW_sh[c], "predb": predb_sh[c],
            "ug2_idx": d_ug2.idx[c], "ug2_dstl": d_ug2.dstl[c],
            "ug2_wv": d_ug2.wv[c],
        }
        for d in (d_iu, d_ui, d_ug1, d_gi1):
            mp[f"{d.name}_tb"] = d.tb[c]
            mp[f"{d.name}_dstl"] = d.dstl[c]
            mp[f"{d.name}_wv"] = d.wv[c]
        in_maps.append(mp)
    return in_maps, {"iu": d_iu, "ui": d_ui, "ug1": d_ug1, "gi1": d_gi1,
                     "ug2": d_ug2}


def _build(struct):
    d_iu, d_ui = struct["iu"], struct["ui"]
    d_ug1, d_gi1 = struct["ug1"], struct["gi1"]
    d_ug2 = struct["ug2"]
    nc = bacc.Bacc("TRN2", target_bir_lowering=False)
    bf = mybir.dt.bfloat16
    f32 = mybir.dt.float32
    i16 = mybir.dt.int16

    P = {}

    def param(name, shape, dt):
        P[name] = nc.declare_dram_parameter(name, list(shape), dt,
                                            isOutput=False)
        return P[name]

    wts = param("wts", [9, 128, 128], bf)
    biases = param("biases", [128, 4], f32)
    ident_d = param("ident", [128, 128], bf)
    iota_d = param("iota", [128, 128], bf)
    hu0T_d = param("hu0T", [128, USH_P], bf)
    hi0T_d = param("hi0T", [128, ISH_P], bf)
    agi_d = param("agi", [ISH_P, NG_P], bf)
    predw = param("predw", [H, ISH_P], bf)
    predb = param("predb", [N_IST, 128], f32)
    for d in (d_iu, d_ui, d_ug1, d_gi1):
        C = d.total_chunks
        param(f"{d.name}_tb", [128, C, H], bf)
        param(f"{d.name}_dstl", [128, C], bf)
        param(f"{d.name}_wv", [128, C], bf)
    C2 = d_ug2.total_chunks
    param("ug2_idx", [128, C2 * 8], i16)
    param("ug2_dstl", [128, C2], bf)
    param("ug2_wv", [128, C2], bf)
    outp = nc.declare_dram_parameter("out", [ISH_P, NG], bf, isOutput=True)

    with tile.TileContext(nc) as tc:
        with (
            tc.tile_pool(name="cst", bufs=1) as cst,
            tc.tile_pool(name="gp", bufs=3) as gp,
            tc.tile_pool(name="sp", bufs=3) as sp,
            tc.tile_pool(name="st", bufs=2) as stp,
            tc.tile_pool(name="psum", bufs=1, space="PSUM") as psum,
            tc.tile_pool(name="dram", bufs=1, space="DRAM") as dram,
        ):
            wt_sb = []
            for k in range(9):
                t = cst.tile([128, 128], bf, tag=f"w{k}")
                nc.sync.dma_start(t[:], wts[k])
                wt_sb.append(t)
            (W_ou_a, W_ou_d, W_oi_a, W_oi_d, W_og1_u, W_og1_i,
             W_og2_u, W_og2_i, W_og2_d) = wt_sb
            bias_sb = cst.tile([128, 4], f32, tag="bias")
            nc.sync.dma_start(bias_sb[:], biases[:])
            ident_sb = cst.tile([128, 128], bf, tag="ident")
            nc.sync.dma_start(ident_sb[:], ident_d[:])
            iota_sb = cst.tile([128, 128], bf, tag="iota")
            nc.sync.dma_start(iota_sb[:], iota_d[:])
            hu0T_sb = cst.tile([128, USH_P], bf, tag="hu0T")
            nc.sync.dma_start(hu0T_sb[:], hu0T_d[:])
            hi0T_sb = cst.tile([128, ISH_P], bf, tag="hi0T")
            nc.sync.dma_start(hi0T_sb[:], hi0T_d[:])
            predb_sb = cst.tile([128, N_IST], f32, tag="predb")
            nc.sync.dma_start(predb_sb[:], predb[:].rearrange("a b -> b a"))

            darr = {}
            for d in (d_iu, d_ui, d_ug1, d_gi1):
                C = d.total_chunks
                td = cst.tile([128, C], bf, tag=f"{d.name}_dstl")
                nc.sync.dma_start(td[:], P[f"{d.name}_dstl"][:])
                tw = cst.tile([128, C], bf, tag=f"{d.name}_wv")
                nc.sync.dma_start(tw[:], P[f"{d.name}_wv"][:])
                darr[d.name] = (td, tw)
            g_idx = cst.tile([128, C2 * 8], i16, tag="ug2_idx")
            nc.sync.dma_start(g_idx[:], P["ug2_idx"][:])
            g_dstl = cst.tile([128, C2], bf, tag="ug2_dstl")
            nc.sync.dma_start(g_dstl[:], P["ug2_dstl"][:])
            g_wv = cst.tile([128, C2], bf, tag="ug2_wv")
            nc.sync.dma_start(g_wv[:], P["ug2_wv"][:])

            ogT = cst.tile([128, 2 * NG_P], bf, tag="ogT")
            hg1T = cst.tile([128, NG_P], bf, tag="hg1T")
            repT = cst.tile([128, NG_P], bf, tag="repT")
            hi1_sb = cst.tile([128, N_IST, 128], bf, tag="hi1")
            aggu_sb = cst.tile([128, N_GT, 128], bf, tag="aggu")

            hu1t = dram.tile([USH_P, H], bf)
            ar_in = dram.tile([128, 2 * NG_P], bf)
            ar_out = dram.tile([128, 2 * NG_P], bf)

            def make_onehot(dstl_sb, wv_sb, cs, cn):
                iota_b = (iota_sb[:].rearrange("p (o k) -> p o k", o=1)
                          .to_broadcast((128, cn, 128)))
                dstl_b = (dstl_sb[:, cs:cs + cn]
                          .rearrange("p (c o) -> p c o", o=1)
                          .to_broadcast((128, cn, 128)))
                wv_b = (wv_sb[:, cs:cs + cn]
                        .rearrange("p (c o) -> p c o", o=1)
                        .to_broadcast((128, cn, 128)))
                ohq = gp.tile([128, SEG, 128], bf, tag="ohq", bufs=2)
                nc.vector.tensor_tensor(ohq[:, :cn, :], iota_b, dstl_b,
                                        AluOpType.is_equal)
                oh = gp.tile([128, SEG, 128], bf, tag="oh")
                nc.vector.tensor_tensor(oh[:, :cn, :], ohq[:, :cn, :],
                                        wv_b, AluOpType.mult)
                return oh

            def stream(d, finish_cb):
                """Stream an SDir's table; accumulate per-dst-tile psum;
                call finish_cb(ti, ps) when a tile completes."""
                dstl_sb, wv_sb = darr[d.name]
                open_ps = {}
                for (cs, cn, pieces) in d.segments:
                    gt = gp.tile([128, SEG, 128], bf, tag="gath")
                    nc.sync.dma_start(gt[:, :cn, :],
                                      P[f"{d.name}_tb"][:, cs:cs + cn, :])
                    oh = make_onehot(dstl_sb, wv_sb, cs, cn)
                    for (ti, lc0, nct, done, total) in pieces:
                        if ti in open_ps:
                            ps = open_ps[ti]
                        else:
                            ps = psum.tile([128, 128], f32, tag="psA",
                                           bufs=2)
                            open_ps[ti] = ps
                        for j in range(nct):
                            nc.tensor.matmul(ps[:], gt[:, lc0 + j, :],
                                             oh[:, lc0 + j, :],
                                             start=(done + j == 0),
                                             stop=(done + j == total - 1))
                        if done + nct == total:
                            del open_ps[ti]
                            finish_cb(ti, ps)

            # ---------- P1: i2u + dense -> hu1 (DRAM table) ----------
            hu_stage = [None]

            def fin_iu(ti, ps):
                aggT = sp.tile([128, 128], bf, tag="aggT", bufs=4)
                nc.scalar.activation(aggT[:], ps[:],
                                     mybir.ActivationFunctionType.Copy)
                pw = psum.tile([128, 128], f32, tag="psW", bufs=2)
                nc.tensor.matmul(pw[:], W_ou_a[:], aggT[:], start=True,
                                 stop=False)
                nc.tensor.matmul(pw[:], W_ou_d[:],
                                 hu0T_sb[:, ti * 128:(ti + 1) * 128],
                                 start=False, stop=True)
                ouT = sp.tile([128, 128], bf, tag="ouT", bufs=4)
                nc.scalar.activation(ouT[:], pw[:],
                                     mybir.ActivationFunctionType.Relu,
                                     bias=bias_sb[:, 1:2])
                ptr = psum.tile([128, 128], bf, tag="psW", bufs=2)
                nc.tensor.transpose(ptr[:], ouT[:], ident_sb[:])
                g, s = ti // 16, ti % 16
                if hu_stage[0] is None:
                    hu_stage[0] = stp.tile([128, 16, 128], bf, tag="hust",
                                           name="hust")
                nc.vector.tensor_copy(hu_stage[0][:, s, :], ptr[:])
                if s == 15 or ti == N_UT - 1:
                    n_g = s + 1
                    nc.sync.dma_start(
                        hu1t[g * 2048:g * 2048 + n_g * 128, :]
                        .rearrange("(k p) h -> p k h", p=128),
                        hu_stage[0][:, :n_g, :])
                    hu_stage[0] = None

            stream(d_iu, fin_iu)

            # ---------- P2: u2i + dense -> hi1 (SBUF, item-major) ----------
            def fin_ui(ti, ps):
                aggT = sp.tile([128, 128], bf, tag="aggT", bufs=4)
                nc.scalar.activation(aggT[:], ps[:],
                                     mybir.ActivationFunctionType.Copy)
                pw = psum.tile([128, 128], f32, tag="psW", bufs=2)
                nc.tensor.matmul(pw[:], W_oi_a[:], aggT[:], start=True,
                                 stop=False)
                nc.tensor.matmul(pw[:], W_oi_d[:],
                                 hi0T_sb[:, ti * 128:(ti + 1) * 128],
                                 start=False, stop=True)
                oiT = sp.tile([128, 128], bf, tag="ouT", bufs=4)
                nc.scalar.activation(oiT[:], pw[:],
                                     mybir.ActivationFunctionType.Relu,
                                     bias=bias_sb[:, 3:4])
                ptr = psum.tile([128, 128], bf, tag="psW", bufs=2)
                nc.tensor.transpose(ptr[:], oiT[:], ident_sb[:])
                nc.vector.tensor_copy(hi1_sb[:, ti, :], ptr[:])

            stream(d_ui, fin_ui)

            # ---------- P3: u2g layer1 -> stash aggT_u per gtile ----------
            def fin_ug1(ti, ps):
                nc.scalar.activation(aggu_sb[:, ti, :], ps[:],
                                     mybir.ActivationFunctionType.Copy)

            stream(d_ug1, fin_ug1)

            # ---------- P4: i2g layer1 + fold -> og1 ----------
            def fin_gi1(ti, ps):
                aggT = sp.tile([128, 128], bf, tag="aggT", bufs=4)
                nc.scalar.activation(aggT[:], ps[:],
                                     mybir.ActivationFunctionType.Copy)
                pw = psum.tile([128, 128], f32, tag="psW", bufs=2)
                nc.tensor.matmul(pw[:], W_og1_u[:], aggu_sb[:, ti, :],
                                 start=True, stop=False)
                nc.tensor.matmul(pw[:], W_og1_i[:], aggT[:],
                                 start=False, stop=True)
                nc.scalar.activation(ogT[:, ti * 128:(ti + 1) * 128], pw[:],
                                     mybir.ActivationFunctionType.Copy)

            stream(d_gi1, fin_gi1)

            # ---------- P5: i2g layer2 dense -> og2 (agi part) ----------
            for jb in range(NG_P // 512):
                pb = psum.tile([128, 512], f32, tag="psB", bufs=2)
                for t in range(N_IST):
                    asb = sp.tile([128, 512], bf, tag="agisb", bufs=3)
                    nc.sync.dma_start(
                        asb[:], agi_d[t * 128:(t + 1) * 128,
                                      jb * 512:(jb + 1) * 512])
                    nc.tensor.matmul(pb[:], hi1_sb[:, t, :], asb[:],
                                     start=(t == 0), stop=(t == N_IST - 1))
                for k in range(4):
                    a0 = sp.tile([128, 128], bf, tag="aggT", bufs=4)
                    nc.scalar.activation(a0[:], pb[:, k * 128:(k + 1) * 128],
                                         mybir.ActivationFunctionType.Copy)
                    pw2 = psum.tile([128, 128], f32, tag="psW", bufs=2)
                    nc.tensor.matmul(pw2[:], W_og2_i[:], a0[:],
                                     start=True, stop=True)
                    sl = slice(NG_P + jb * 512 + k * 128,
                               NG_P + jb * 512 + (k + 1) * 128)
                    nc.scalar.activation(ogT[:, sl], pw2[:],
                                         mybir.ActivationFunctionType.Copy)

            # ---------- P6: u2g layer2 gather -> og2 += W @ agg ----------
            for (cs, cn, seg_tiles) in d_ug2.segments:
                gt = gp.tile([128, SEG_G, 128], bf, tag="gath")
                n_idx = cn * 128
                nc.gpsimd.dma_gather(
                    gt[:, :cn, :], hu1t[:],
                    g_idx[:, cs * 8:(cs + cn) * 8],
                    n_idx, n_idx, H, elem_step=H, single_packet=False)
                oh = make_onehot(g_dstl, g_wv, cs, cn)
                for (ti, ofs_t, nct) in seg_tiles:
                    lc0 = ofs_t - cs
                    ps = psum.tile([128, 128], f32, tag="psA", bufs=2)
                    for j in range(nct):
                        nc.tensor.matmul(ps[:], gt[:, lc0 + j, :],
                                         oh[:, lc0 + j, :],
                                         start=(j == 0), stop=(j == nct - 1))
                    aggT = sp.tile([128, 128], bf, tag="aggT", bufs=4)
                    nc.scalar.activation(aggT[:], ps[:],
                                         mybir.ActivationFunctionType.Copy)
                    pw = psum.tile([128, 128], f32, tag="psW", bufs=2)
                    nc.tensor.matmul(pw[:], W_og2_u[:], aggT[:],
                                     start=True, stop=True)
                    sl = slice(NG_P + ti * 128, NG_P + (ti + 1) * 128)
                    nc.vector.tensor_tensor(ogT[:, sl], ogT[:, sl], pw[:],
                                            AluOpType.add)

            # ---------- P7: AllReduce [og1 | og2] ----------
            nc.sync.dma_start(ar_in[:], ogT[:])
            nc.gpsimd.collective_compute(
                "AllReduce", AluOpType.add,
                replica_groups=[list(range(W))],
                ins=[ar_in.opt()], outs=[ar_out.opt()])
            ar_sb = ogT
            nc.sync.dma_start(ar_sb[:], ar_out[:])
            nc.scalar.activation(hg1T[:], ar_sb[:, 0:NG_P],
                                 mybir.ActivationFunctionType.Relu,
                                 bias=bias_sb[:, 0:1])
            for j in range(NG_P // 512):
                pf = psum.tile([128, 512], f32, tag="psB", bufs=2)
                nc.tensor.matmul(pf[:], W_og2_d[:],
                                 hg1T[:, j * 512:(j + 1) * 512],
                                 start=True, stop=True)
                tt = sp.tile([128, 512], bf, tag="o2t")
                nc.vector.tensor_tensor(
                    tt[:], ar_sb[:, NG_P + j * 512:NG_P + (j + 1) * 512],
                    pf[:], AluOpType.add)
                nc.scalar.activation(repT[:, j * 512:(j + 1) * 512], tt[:],
                                     mybir.ActivationFunctionType.Relu,
                                     bias=bias_sb[:, 2:3])

            # ---------- P8: out[item, group] = predW.T @ repT + b ----------
            for t in range(N_IST):
                pw_t = sp.tile([H, 128], bf, tag="pwt")
                nc.sync.dma_start(pw_t[:], predw[:, t * 128:(t + 1) * 128])
                for j in range((NG + 1023) // 1024):
                    wj = min(1024, NG - j * 1024)
                    stg = stp.tile([128, 1024], bf, tag="fstage", bufs=3)
                    for q in range((wj + 511) // 512):
                        wq = min(512, wj - q * 512)
                        col = j * 1024 + q * 512
                        pf = psum.tile([128, 512], f32, tag="psB", bufs=2)
                        nc.tensor.matmul(
                            pf[:, :wq], pw_t[:],
                            repT[:, col:col + wq], start=True, stop=True)
                        nc.vector.tensor_scalar(
                            stg[:, q * 512:q * 512 + wq], pf[:, :wq],
                            predb_sb[:, t:t + 1], None, AluOpType.add)
                    nc.sync.dma_start(
                        outp[t * 128:(t + 1) * 128, j * 1024:j * 1024 + wj],
                        stg[:, :wj])
    nc.compile()
    return nc


def kernel(**inputs):
    in_maps, struct = _prep(inputs)
    nc = _build(struct)
    res = run_bass_kernel_spmd(nc, in_maps, list(range(W)))
    parts = [res.results[c]["out"][:ISH] for c in range(W)]
    full = np.concatenate(parts, axis=0).astype(np.float32)  # [NI, NG]
    return full.T  # [NG, NI] zero-copy view
